# revision 25
# baseline (speedup 1.0000x reference)
"""Trainium2 Bass kernel for nn_AttentionMLP (pairwise-MLP attention + softmax).

Math (per batch b):
  hA = inputA[b] @ W1[:128]          # (K, H)
  hB = inputB[b] @ W1[128:]          # (L, H)
  scores[k, l] = sum_h relu(hA[k, h] + hB[l, h] + b1[h]) * w2[h]
  out[b, k, :] = softmax(scores[k, :])

Shapes: B=2, K=128, L=4096, D=H=128.

Distribution: pure data parallel over the (b, k) grid — core c handles
b = c // 4 and a 32-wide k block (no collectives; the softmax over L is
core-local).

Per-core device algorithm (SBUF partition axis = H):
  1. hBT = W1b.T @ inputB[b].T -> PSUM in 512-col chunks (inputs
     pre-transposed on host so the contraction lands on partitions).
     Each PSUM chunk is consumed by ScalarE: one plain Copy evacuation
     into the shared bf16 hBT SBUF tile, plus one FUSED relu+bias
     activation per ACT-assigned k writing that k's R tile directly.
     Measured HW quirk this exploits: an ACT activation whose main
     operand is SBUF *and* carries a per-partition bias AP serializes
     against DVE's 4x-mode passes (the bias read takes the shared
     DVE/GpSimd SBUF port pair) — but with the main operand in PSUM the
     bias uses ACT's own SBUF read port and the engines fully overlap.
  2. bias[:, k] = W1a.T @ inputA[b].T + b1  (fp32 [128, 32])
  3. For the remaining k's: R_k = relu(hBT + bias[:, k]) as one
     [128, 4096] bf16 DVE tensor_scalar pass (add+max, 4x mode,
     ~1.12us measured). GpSimd is useless here (~70us/pass measured).
  4. scores = w2.T @ R_k via M=32 matmuls (N=512 chunks) whose weight
     matrix is a 32-wide slice of a zeros|w2|zeros band — w2 lands in
     column 4*(k%8)+(chunk//2), so chunk c of k accumulates into PSUM
     partition 4*k + c//2, columns 512*(c%2):...  All 256 matmuls
     accumulate into ONE [128, 1024] PSUM tile (2 banks) holding the
     scores in softmax layout: partition 4k+q = l-range [1024q:1024(q+1)).
     Four matmuls run concurrently via PE col-tiling (tile_position=
     (0,32j), k's 8 apart), so PE streams ~4 cols/cycle.
  5. Softmax without max-subtraction (scores are O(1)): ScalarE exp reads
     the PSUM tile directly (this is also the PSUM evacuation) with
     accum_out producing fp32 row sums; per-k sums = quarter sums
     combined and broadcast back via tiny 0/1 matmuls; final scale via
     ACT Copy+scale into bf16 and per-half output DMA (host upcasts).

Timing-loop structure (software-pipelined, measured steady state): each
loop half-body emits, in order,
  [produce-head i+1 (bias mm+add) | hBT chunks 0-1 (i+1) | group 0 (i)
   | exp+sum-combine of i-1 | chunks 2-3 | group 1 | chunks 4-5
   | group 2 | recip+bcast of i-1 | chunks 6-7 | groups 3-7
   | final muls + out DMA of i-1]
so every engine queue is gap-free: ACT spends the iteration on exp +
chunk evac/fused-relu, DVE on its 26 passes, PE on scoring with the
produce matmuls slotted between groups at the pace ACT consumes PSUM
chunks. Tiles double-buffer via tag rotation (bufs=2 pools, two
half-body calls per For_i trace); the pre/post halves outside the loop
cancel in the (T2-T1)/(N2-N1) timing difference.
"""

import os
import sys

for _p in ("/opt/trn_rl_repo", "/root/.axon_site/_ro/trn_rl_repo"):
    if os.path.isdir(_p) and _p not in sys.path:
        sys.path.insert(0, _p)

import numpy as np
import ml_dtypes

BF = ml_dtypes.bfloat16
B, K, L, D, H = 2, 128, 4096, 128, 128
NCORES = 8
KPC = 32   # k's per core
NG = 8     # concurrency groups; group g = k's {g, 8+g, 16+g, 24+g}

import json as _json


def _env_set(name, default):
    v = os.environ.get(name)
    if v:
        return {tuple(p) if isinstance(p, list) else p for p in _json.loads(v)}
    return default


# (g, j) passes produced by ACT fused relu+bias PSUM evacuation
FUSED_PASSES = _env_set("KERNEL_FUSED", {(g, 2) for g in range(6)})
# (g, j) passes as plain ACT SBUF activations (serialize vs DVE 4x; avoid)
ACT_PASSES = _env_set("KERNEL_ACT_PASSES", set())
# remaining passes run on DVE
# logical iterations per For_i body
LOOP_HALVES = int(os.environ.get("KERNEL_LOOP_HALVES", "2"))
STAGGERED = os.environ.get("KERNEL_STAGGERED", "0") == "1"

_BUILT = None


class _Ctx:
    def __init__(self, nc, tc, pools, aps, types):
        self.nc, self.tc = nc, tc
        (self.work, self.rpool, self.apool, self.stage, self.psum,
         self.epsum) = pools
        (self.xbt, self.out, self.w1a_sb, self.w1b_sb, self.xat_sb,
         self.b1_sb, self.wband_sb, self.wcomb_sb, self.wbcast_sb) = aps
        (self.f32, self.bf, self.AF, self.ALU) = types


def _produce_head(cx):
    """bias matrix + tile allocations + xbt chunk DMAs for one logical
    iteration; hBT chunk matmuls/evacs are emitted via _produce_chunks."""
    nc, f32, bf, ALU = cx.nc, cx.f32, cx.bf, cx.ALU
    ps_h = cx.psum.tile([128, 512], f32, tag="ps")
    nc.tensor.matmul(ps_h[:, 0:KPC], lhsT=cx.w1a_sb[:], rhs=cx.xat_sb[:],
                     start=True, stop=True)
    bias_sb = cx.work.tile([128, KPC], f32, tag="bias")
    nc.vector.tensor_scalar(out=bias_sb[:], in0=ps_h[:, 0:KPC],
                            scalar1=cx.b1_sb[:, 0:1], scalar2=None,
                            op0=ALU.add)
    hbt_sb = cx.work.tile([128, L], bf, tag="hbt")
    fused = {gj: cx.apool.tile([128, L], bf, tag="rf", name="rf")
             for gj in FUSED_PASSES}
    stages = []
    for c in range(8):
        st = cx.stage.tile([128, 512], bf, tag="xc", name="xc")
        nc.sync.dma_start(st[:], cx.xbt[:, 512 * c:512 * c + 512])
        stages.append(st)
    return {"hbt": hbt_sb, "bias": bias_sb, "fused": fused, "stages": stages}


def _produce_chunks(cx, cur, c0, c1):
    """hBT chunk matmuls + ScalarE evacuations (plain copy into hbt, plus
    fused relu+bias into each ACT-assigned k's R tile) for chunks c0..c1."""
    nc, f32, AF = cx.nc, cx.f32, cx.AF
    for c in range(c0, c1 + 1):
        ps_c = cx.psum.tile([128, 512], f32, tag="ps")
        sl = slice(512 * c, 512 * c + 512)
        nc.tensor.matmul(ps_c[:], lhsT=cx.w1b_sb[:], rhs=cur["stages"][c][:],
                         start=True, stop=True)
        nc.scalar.copy(cur["hbt"][:, sl], ps_c[:])
        for (g, j), rt in cur["fused"].items():
            k = 8 * j + g
            nc.scalar.activation(rt[:, sl], ps_c[:], AF.Relu,
                                 bias=cur["bias"][:, k:k + 1], scale=1.0)


def _emit_group(cx, cur, g, e_ps, split_g0=False):
    """relu passes (non-fused) + 32 scoring matmuls for group g."""
    nc, bf, AF, ALU = cx.nc, cx.bf, cx.AF, cx.ALU
    hbt_sb, bias_sb = cur["hbt"], cur["bias"]
    rts = []
    for j in range(4):
        k = 8 * j + g
        if (g, j) in FUSED_PASSES:
            rts.append(cur["fused"][(g, j)])
            continue
        rt = cx.rpool.tile([128, L], bf, tag="r")
        if (g, j) in ACT_PASSES:
            parts = ((0, 4096, "act"),)
        elif split_g0 and g == 0:
            # halves so the first half only needs hbt chunks 0-3
            # (single-shot startup)
            parts = ((0, 2048, "dve"), (2048, 4096, "dve"))
        else:
            parts = ((0, 4096, "dve"),)
        for lo, hi, eng in parts:
            if eng == "act":
                nc.scalar.activation(rt[:, lo:hi], hbt_sb[:, lo:hi], AF.Relu,
                                     bias=bias_sb[:, k:k + 1], scale=1.0)
            else:
                nc.vector.tensor_scalar(
                    out=rt[:, lo:hi], in0=hbt_sb[:, lo:hi],
                    scalar1=bias_sb[:, k:k + 1], scalar2=0.0,
                    op0=ALU.add, op1=ALU.max)
        rts.append(rt)
    # q-major: one weight slice serves 8 matmuls
    order = [(2 * q + win, j) for q in range(4)
             for j in range(4) for win in range(2)]
    for c, j in order:
        win = c % 2
        v = 4 * g + c // 2  # local column for w2
        nc.tensor.matmul(
            e_ps[32 * j:32 * j + 32, 512 * win:512 * win + 512],
            lhsT=cx.wband_sb[:, 31 - v:63 - v],
            rhs=rts[j][:, 512 * c:512 * c + 512],
            start=(g == 0 and c // 2 == 0),
            stop=(g == NG - 1 and c // 2 == 3),
            tile_position=(0, 32 * j),
            skip_group_check=True)


def _tail_exp(cx, st):
    """exp + row sums + sum-combine matmuls for scoring state `st`."""
    nc, f32, bf, AF = cx.nc, cx.f32, cx.bf, cx.AF
    e_ps = st["eps"]
    e2_sb = cx.work.tile([128, 1024], bf, tag="exp")
    s0_sb = cx.work.tile([128, 1], f32, tag="sums0")
    s1_sb = cx.work.tile([128, 1], f32, tag="sums1")
    nc.scalar.activation(e2_sb[:, 0:512], e_ps[:, 0:512], AF.Exp,
                         accum_out=s0_sb[:, 0:1])
    nc.scalar.activation(e2_sb[:, 512:1024], e_ps[:, 512:1024],
                         AF.Exp, accum_out=s1_sb[:, 0:1])
    ps_t = cx.psum.tile([128, 512], f32, tag="ps")
    nc.tensor.matmul(ps_t[0:KPC, 0:1], lhsT=cx.wcomb_sb[:],
                     rhs=s0_sb[:, 0:1], start=True, stop=False)
    nc.tensor.matmul(ps_t[0:KPC, 0:1], lhsT=cx.wcomb_sb[:],
                     rhs=s1_sb[:, 0:1], start=False, stop=True)
    st["e2"], st["ps_t"] = e2_sb, ps_t


def _tail_recip(cx, st):
    """reciprocal + broadcast matmul + SBUF staging of the scale column."""
    nc, f32 = cx.nc, cx.f32
    tr_sb = cx.work.tile([KPC, 1], f32, tag="recip")
    nc.vector.reciprocal(tr_sb[:], st["ps_t"][0:KPC, 0:1])
    ps_u = cx.psum.tile([128, 512], f32, tag="ps")
    nc.tensor.matmul(ps_u[:, 0:1], lhsT=cx.wbcast_sb[:], rhs=tr_sb[:],
                     start=True, stop=True)
    sc_sb = cx.work.tile([128, 1], f32, tag="scale")
    nc.vector.tensor_copy(sc_sb[:], ps_u[:, 0:1])
    st["sc"] = sc_sb


def _tail_muls(cx, st):
    """final softmax scale (ACT Copy+scale, bf16) + output DMA."""
    nc, bf, AF = cx.nc, cx.bf, cx.AF
    f_sb = cx.work.tile([128, 1024], bf, tag="final")
    nc.scalar.activation(f_sb[:, 0:512], st["e2"][:, 0:512], AF.Copy,
                         scale=st["sc"][:, 0:1])
    nc.sync.dma_start(cx.out[:, 0:512], f_sb[:, 0:512])
    nc.scalar.activation(f_sb[:, 512:1024], st["e2"][:, 512:1024], AF.Copy,
                         scale=st["sc"][:, 0:1])
    nc.sync.dma_start(cx.out[:, 512:1024], f_sb[:, 512:1024])


def _emit_tail(cx, st):
    _tail_exp(cx, st)
    _tail_recip(cx, st)
    _tail_muls(cx, st)


def _emit_half(cx, cur, prev_st, produce_next):
    """Pipelined half-body: passes+scoring of `cur`, tail of `prev_st`,
    produce of the next logical iteration. Returns (state, nxt)."""
    f32 = cx.f32
    e_ps = cx.epsum.tile([128, 1024], f32, tag="eps")
    st = {"eps": e_ps}
    nxt = _produce_head(cx) if produce_next else None
    if nxt is not None:
        _produce_chunks(cx, nxt, 0, 1)
    _emit_group(cx, cur, 0, e_ps)
    if prev_st is not None:
        _tail_exp(cx, prev_st)
    if nxt is not None:
        _produce_chunks(cx, nxt, 2, 3)
    _emit_group(cx, cur, 1, e_ps)
    if nxt is not None:
        _produce_chunks(cx, nxt, 4, 5)
    _emit_group(cx, cur, 2, e_ps)
    if prev_st is not None:
        _tail_recip(cx, prev_st)
    if nxt is not None:
        _produce_chunks(cx, nxt, 6, 7)
    for g in range(3, NG):
        _emit_group(cx, cur, g, e_ps)
    if prev_st is not None:
        _tail_muls(cx, prev_st)
    return st, nxt


def _body_straight(cx):
    """Single-shot body: produce + groups (g0 split for startup) + tail."""
    cur = _produce_head(cx)
    _produce_chunks(cx, cur, 0, 7)
    f32 = cx.f32
    e_ps = cx.epsum.tile([128, 1024], f32, tag="eps")
    for g in range(NG):
        _emit_group(cx, cur, g, e_ps, split_g0=True)
    _emit_tail(cx, {"eps": e_ps})


def _build(reps=1, loop=False):
    import concourse.mybir as mybir
    import concourse.tile as tile
    from concourse import bacc

    dt = mybir.dt
    f32, bf = dt.float32, dt.bfloat16
    AF = mybir.ActivationFunctionType
    ALU = mybir.AluOpType

    nc = bacc.Bacc("TRN2", target_bir_lowering=False, debug=False,
                   enable_asserts=True)

    xbt = nc.dram_tensor("xbt", [128, L], bf, kind="ExternalInput").ap()
    xat = nc.dram_tensor("xat", [128, KPC], bf, kind="ExternalInput").ap()
    w1a = nc.dram_tensor("w1a", [128, H], bf, kind="ExternalInput").ap()
    w1b = nc.dram_tensor("w1b", [128, H], bf, kind="ExternalInput").ap()
    b1c = nc.dram_tensor("b1c", [128, 1], f32, kind="ExternalInput").ap()
    wband = nc.dram_tensor("wband", [128, 64], bf, kind="ExternalInput").ap()
    wcomb = nc.dram_tensor("wcomb", [128, KPC], f32, kind="ExternalInput").ap()
    wbcast = nc.dram_tensor("wbcast", [KPC, 128], f32, kind="ExternalInput").ap()
    out = nc.dram_tensor("out", [128, 1024], bf, kind="ExternalOutput").ap()

    with tile.TileContext(nc) as tc:
        with (
            tc.tile_pool(name="consts", bufs=1) as consts,
            tc.tile_pool(name="work", bufs=2) as work,
            tc.tile_pool(name="rpool", bufs=11) as rpool,
            tc.tile_pool(name="apool",
                         bufs=max(1, 2 * len(FUSED_PASSES))) as apool,
            tc.tile_pool(name="stage", bufs=9) as stage,
            tc.tile_pool(name="psum", bufs=3, space="PSUM") as psum,
            tc.tile_pool(name="epsum", bufs=2, space="PSUM") as epsum,
        ):
            w1a_sb = consts.tile([128, H], bf, tag="w1a")
            nc.sync.dma_start(w1a_sb[:], w1a)
            w1b_sb = consts.tile([128, H], bf, tag="w1b")
            nc.sync.dma_start(w1b_sb[:], w1b)
            xat_sb = consts.tile([128, KPC], bf, tag="xat")
            nc.sync.dma_start(xat_sb[:], xat)
            b1_sb = consts.tile([128, 1], f32, tag="b1")
            nc.sync.dma_start(b1_sb[:], b1c)
            wband_sb = consts.tile([128, 64], bf, tag="wband")
            nc.sync.dma_start(wband_sb[:], wband)
            wcomb_sb = consts.tile([128, KPC], f32, tag="wcomb")
            nc.sync.dma_start(wcomb_sb[:], wcomb)
            wbcast_sb = consts.tile([KPC, 128], f32, tag="wbcast")
            nc.sync.dma_start(wbcast_sb[:], wbcast)
            # dummy ACT op issued first so the ~2.7us activation-table load
            # overlaps the input DMAs; Exp anchors the exp_and_others table
            # set which also holds Relu and Copy
            warm_sb = consts.tile([128, 1], f32, tag="warm")
            nc.vector.memset(warm_sb[:], 0.0)
            nc.scalar.activation(warm_sb[:], warm_sb[:], AF.Exp)

            cx = _Ctx(nc, tc,
                      (work, rpool, apool, stage, psum, epsum),
                      (xbt, out, w1a_sb, w1b_sb, xat_sb, b1_sb, wband_sb,
                       wcomb_sb, wbcast_sb),
                      (f32, bf, AF, ALU))

            if loop and reps > 1:
                U = LOOP_HALVES
                assert U % 2 == 0 and (reps - 2) % U == 0 and reps >= U + 2, \
                    f"loop path needs reps = 2 + {U}*m"
                m = (reps - 2) // U
                cur0 = _produce_head(cx)                     # logical 0
                _produce_chunks(cx, cur0, 0, 7)
                st_a, cur1 = _emit_half(cx, cur0, None, True)
                with tc.For_i(0, m, 1, staggered_reset=STAGGERED):
                    st, cur = st_a, cur1
                    for _u in range(U):
                        st, cur = _emit_half(cx, cur, st, True)
                st_last, _ = _emit_half(cx, cur1, st_a, False)
                _emit_tail(cx, st_last)
            else:
                for _rep in range(reps):
                    _body_straight(cx)

    nc.compile()
    return nc


def _get_built():
    global _BUILT
    if _BUILT is None:
        _BUILT = _build()
    return _BUILT


def make_in_maps(inputA, inputB, W1, b1, w2):
    wband = np.zeros((128, 64), np.float32)
    wband[:, 31] = w2
    wcomb = (np.arange(128)[:, None] // 4 == np.arange(KPC)[None, :]) \
        .astype(np.float32)
    wbcast = (np.arange(128)[None, :] // 4 == np.arange(KPC)[:, None]) \
        .astype(np.float32)
    w1a = np.ascontiguousarray(W1[:D]).astype(BF)
    w1b = np.ascontiguousarray(W1[D:]).astype(BF)
    b1c = np.ascontiguousarray(b1.reshape(128, 1)).astype(np.float32)
    wband = wband.astype(BF)
    in_maps = []
    for core in range(NCORES):
        b, kq = core // 4, core % 4
        k0 = KPC * kq
        in_maps.append({
            "xbt": np.ascontiguousarray(inputB[b].T).astype(BF),
            "xat": np.ascontiguousarray(inputA[b, k0:k0 + KPC].T).astype(BF),
            "w1a": w1a, "w1b": w1b, "b1c": b1c, "wband": wband,
            "wcomb": wcomb, "wbcast": wbcast,
        })
    return in_maps


def assemble(results):
    """results: list of 8 dicts with 'out' [128, 1024] bf16."""
    full = np.empty((B, K, L), np.float32)
    for core in range(NCORES):
        b, kq = core // 4, core % 4
        full[b, KPC * kq:KPC * (kq + 1)] = \
            np.asarray(results[core]["out"]).astype(np.float32) \
            .reshape(KPC, L)
    return full


def kernel(**inputs):
    from concourse.bass_utils import run_bass_kernel_spmd

    inputA = np.asarray(inputs["inputA"], np.float32)
    inputB = np.asarray(inputs["inputB"], np.float32)
    W1 = np.asarray(inputs["W1"], np.float32)
    b1 = np.asarray(inputs["b1"], np.float32)
    w2 = np.asarray(inputs["w2"], np.float32)

    nc = _get_built()
    in_maps = make_in_maps(inputA, inputB, W1, b1, w2)
    res = run_bass_kernel_spmd(nc, in_maps, core_ids=list(range(NCORES)))
    return assemble(res.results)


# revision 26
# speedup vs baseline: 1.4034x; 1.4034x over previous
"""Trainium2 Bass kernel for nn_AttentionMLP (pairwise-MLP attention + softmax).

Math (per batch b):
  hA = inputA[b] @ W1[:128]          # (K, H)
  hB = inputB[b] @ W1[128:]          # (L, H)
  scores[k, l] = sum_h relu(hA[k, h] + hB[l, h] + b1[h]) * w2[h]
  out[b, k, :] = softmax(scores[k, :])

Shapes: B=2, K=128, L=4096, D=H=128.

Distribution: pure data parallel over the (b, k) grid — core c handles
b = c // 4 and a 32-wide k block (no collectives; the softmax over L is
core-local).

Per-core device algorithm (SBUF partition axis = H):
  1. hBT = W1b.T @ inputB[b].T -> PSUM in 512-col chunks (inputs
     pre-transposed on host so the contraction lands on partitions).
     Each PSUM chunk is consumed by ScalarE: one plain Copy evacuation
     into the shared bf16 hBT SBUF tile, plus one FUSED relu+bias
     activation per ACT-assigned k writing that k's R tile directly.
     Measured HW quirk this exploits: an ACT activation whose main
     operand is SBUF *and* carries a per-partition bias AP serializes
     against DVE's 4x-mode passes (the bias read takes the shared
     DVE/GpSimd SBUF port pair) — but with the main operand in PSUM the
     bias uses ACT's own SBUF read port and the engines fully overlap.
  2. bias[:, k] = W1a.T @ inputA[b].T + b1  (fp32 [128, 32])
  3. For the remaining k's: R_k = relu(hBT + bias[:, k]) as one
     [128, 4096] bf16 DVE tensor_scalar pass (add+max, 4x mode,
     ~1.12us measured). GpSimd is useless here (~70us/pass measured).
  4. scores = w2.T @ R_k via M=32 matmuls (N=512 chunks) whose weight
     matrix is a 32-wide slice of a zeros|w2|zeros band — w2 lands in
     column 4*(k%8)+(chunk//2), so chunk c of k accumulates into PSUM
     partition 4*k + c//2, columns 512*(c%2):...  All 256 matmuls
     accumulate into ONE [128, 1024] PSUM tile (2 banks) holding the
     scores in softmax layout: partition 4k+q = l-range [1024q:1024(q+1)).
     Four matmuls run concurrently via PE col-tiling (tile_position=
     (0,32j), k's 8 apart), so PE streams ~4 cols/cycle.
  5. Softmax without max-subtraction (scores are O(1)): ScalarE exp reads
     the PSUM tile directly (this is also the PSUM evacuation) with
     accum_out producing fp32 row sums; per-k sums = quarter sums
     combined and broadcast back via tiny 0/1 matmuls; final scale via
     ACT Copy+scale into bf16 and per-half output DMA (host upcasts).

Timing-loop structure (software-pipelined, measured steady state): each
loop half-body emits, in order,
  [produce-head i+1 (bias mm+add) | hBT chunks 0-1 (i+1) | group 0 (i)
   | exp+sum-combine of i-1 | chunks 2-3 | group 1 | chunks 4-5
   | group 2 | recip+bcast of i-1 | chunks 6-7 | groups 3-7
   | final muls + out DMA of i-1]
so every engine queue is gap-free: ACT spends the iteration on exp +
chunk evac/fused-relu, DVE on its 26 passes, PE on scoring with the
produce matmuls slotted between groups at the pace ACT consumes PSUM
chunks. Tiles double-buffer via tag rotation (bufs=2 pools, two
half-body calls per For_i trace); the pre/post halves outside the loop
cancel in the (T2-T1)/(N2-N1) timing difference.
"""

import os
import sys

for _p in ("/opt/trn_rl_repo", "/root/.axon_site/_ro/trn_rl_repo"):
    if os.path.isdir(_p) and _p not in sys.path:
        sys.path.insert(0, _p)

import numpy as np
import ml_dtypes

BF = ml_dtypes.bfloat16
B, K, L, D, H = 2, 128, 4096, 128, 128
NCORES = 8
KPC = 32   # k's per core
NG = 8     # concurrency groups; group g = k's {g, 8+g, 16+g, 24+g}

import json as _json


def _env_set(name, default):
    v = os.environ.get(name)
    if v:
        return {tuple(p) if isinstance(p, list) else p for p in _json.loads(v)}
    return default


# (g, j) passes produced by ACT fused relu+bias PSUM evacuation
FUSED_PASSES = _env_set("KERNEL_FUSED", {(g, 2) for g in range(6)})
# (g, j) passes as plain ACT SBUF activations (serialize vs DVE 4x; avoid)
ACT_PASSES = _env_set("KERNEL_ACT_PASSES", set())
# remaining passes run on DVE
# logical iterations per For_i body
LOOP_HALVES = int(os.environ.get("KERNEL_LOOP_HALVES", "2"))
STAGGERED = os.environ.get("KERNEL_STAGGERED", "0") == "1"

_BUILT = None


class _Ctx:
    def __init__(self, nc, tc, pools, aps, types):
        self.nc, self.tc = nc, tc
        (self.work, self.rpool, self.apool, self.stage, self.psum,
         self.epsum) = pools
        (self.xbt, self.out, self.w1a_sb, self.w1b_sb, self.xat_sb,
         self.b1_sb, self.wband_sb, self.wcomb_sb, self.wbcast_sb) = aps
        (self.f32, self.bf, self.AF, self.ALU) = types


def _produce_head(cx):
    """bias matrix + tile allocations + xbt chunk DMAs for one logical
    iteration; hBT chunk matmuls/evacs are emitted via _produce_chunks."""
    nc, f32, bf, ALU = cx.nc, cx.f32, cx.bf, cx.ALU
    ps_h = cx.psum.tile([128, 512], f32, tag="ps")
    nc.tensor.matmul(ps_h[:, 0:KPC], lhsT=cx.w1a_sb[:], rhs=cx.xat_sb[:],
                     start=True, stop=True)
    bias_sb = cx.work.tile([128, KPC], f32, tag="bias")
    nc.vector.tensor_scalar(out=bias_sb[:], in0=ps_h[:, 0:KPC],
                            scalar1=cx.b1_sb[:, 0:1], scalar2=None,
                            op0=ALU.add)
    hbt_sb = cx.work.tile([128, L], bf, tag="hbt")
    fused = {gj: cx.apool.tile([128, L], bf, tag="rf", name="rf")
             for gj in FUSED_PASSES}
    stages = []
    for c in range(8):
        st = cx.stage.tile([128, 512], bf, tag="xc", name="xc")
        nc.sync.dma_start(st[:], cx.xbt[:, 512 * c:512 * c + 512])
        stages.append(st)
    return {"hbt": hbt_sb, "bias": bias_sb, "fused": fused, "stages": stages}


def _produce_chunks(cx, cur, c0, c1):
    """hBT chunk matmuls + ScalarE evacuations (plain copy into hbt, plus
    fused relu+bias into each ACT-assigned k's R tile) for chunks c0..c1."""
    nc, f32, AF = cx.nc, cx.f32, cx.AF
    for c in range(c0, c1 + 1):
        ps_c = cx.psum.tile([128, 512], f32, tag="ps")
        sl = slice(512 * c, 512 * c + 512)
        nc.tensor.matmul(ps_c[:], lhsT=cx.w1b_sb[:], rhs=cur["stages"][c][:],
                         start=True, stop=True)
        nc.scalar.copy(cur["hbt"][:, sl], ps_c[:])
        for (g, j), rt in cur["fused"].items():
            k = 8 * j + g
            nc.scalar.activation(rt[:, sl], ps_c[:], AF.Relu,
                                 bias=cur["bias"][:, k:k + 1], scale=1.0)


def _emit_group(cx, cur, g, e_ps, split_g0=False):
    """relu passes (non-fused) + 32 scoring matmuls for group g."""
    nc, bf, AF, ALU = cx.nc, cx.bf, cx.AF, cx.ALU
    hbt_sb, bias_sb = cur["hbt"], cur["bias"]
    rts = []
    for j in range(4):
        k = 8 * j + g
        if (g, j) in FUSED_PASSES:
            rts.append(cur["fused"][(g, j)])
            continue
        rt = cx.rpool.tile([128, L], bf, tag="r")
        if (g, j) in ACT_PASSES:
            parts = ((0, 4096, "act"),)
        elif split_g0 and g == 0:
            # halves so the first half only needs hbt chunks 0-3
            # (single-shot startup)
            parts = ((0, 2048, "dve"), (2048, 4096, "dve"))
        else:
            parts = ((0, 4096, "dve"),)
        for lo, hi, eng in parts:
            if eng == "act":
                nc.scalar.activation(rt[:, lo:hi], hbt_sb[:, lo:hi], AF.Relu,
                                     bias=bias_sb[:, k:k + 1], scale=1.0)
            else:
                nc.vector.tensor_scalar(
                    out=rt[:, lo:hi], in0=hbt_sb[:, lo:hi],
                    scalar1=bias_sb[:, k:k + 1], scalar2=0.0,
                    op0=ALU.add, op1=ALU.max)
        rts.append(rt)
    # q-major: one weight slice serves 8 matmuls
    order = [(2 * q + win, j) for q in range(4)
             for j in range(4) for win in range(2)]
    for c, j in order:
        win = c % 2
        v = 4 * g + c // 2  # local column for w2
        nc.tensor.matmul(
            e_ps[32 * j:32 * j + 32, 512 * win:512 * win + 512],
            lhsT=cx.wband_sb[:, 31 - v:63 - v],
            rhs=rts[j][:, 512 * c:512 * c + 512],
            start=(g == 0 and c // 2 == 0),
            stop=(g == NG - 1 and c // 2 == 3),
            tile_position=(0, 32 * j),
            skip_group_check=True)


def _tail_exp(cx, st):
    """exp + row sums + sum-combine matmuls for scoring state `st`."""
    nc, f32, bf, AF = cx.nc, cx.f32, cx.bf, cx.AF
    e_ps = st["eps"]
    e2_sb = cx.work.tile([128, 1024], bf, tag="exp")
    s0_sb = cx.work.tile([128, 1], f32, tag="sums0")
    s1_sb = cx.work.tile([128, 1], f32, tag="sums1")
    nc.scalar.activation(e2_sb[:, 0:512], e_ps[:, 0:512], AF.Exp,
                         accum_out=s0_sb[:, 0:1])
    nc.scalar.activation(e2_sb[:, 512:1024], e_ps[:, 512:1024],
                         AF.Exp, accum_out=s1_sb[:, 0:1])
    ps_t = cx.psum.tile([128, 512], f32, tag="ps")
    nc.tensor.matmul(ps_t[0:KPC, 0:1], lhsT=cx.wcomb_sb[:],
                     rhs=s0_sb[:, 0:1], start=True, stop=False)
    nc.tensor.matmul(ps_t[0:KPC, 0:1], lhsT=cx.wcomb_sb[:],
                     rhs=s1_sb[:, 0:1], start=False, stop=True)
    st["e2"], st["ps_t"] = e2_sb, ps_t


def _tail_recip(cx, st):
    """reciprocal + broadcast matmul + SBUF staging of the scale column."""
    nc, f32 = cx.nc, cx.f32
    tr_sb = cx.work.tile([KPC, 1], f32, tag="recip")
    nc.vector.reciprocal(tr_sb[:], st["ps_t"][0:KPC, 0:1])
    ps_u = cx.psum.tile([128, 512], f32, tag="ps")
    nc.tensor.matmul(ps_u[:, 0:1], lhsT=cx.wbcast_sb[:], rhs=tr_sb[:],
                     start=True, stop=True)
    sc_sb = cx.work.tile([128, 1], f32, tag="scale")
    nc.vector.tensor_copy(sc_sb[:], ps_u[:, 0:1])
    st["sc"] = sc_sb


def _tail_muls(cx, st):
    """final softmax scale (ACT Copy+scale, bf16) + output DMA."""
    nc, bf, AF = cx.nc, cx.bf, cx.AF
    f_sb = cx.work.tile([128, 1024], bf, tag="final")
    nc.scalar.activation(f_sb[:, 0:512], st["e2"][:, 0:512], AF.Copy,
                         scale=st["sc"][:, 0:1])
    nc.sync.dma_start(cx.out[:, 0:512], f_sb[:, 0:512])
    nc.scalar.activation(f_sb[:, 512:1024], st["e2"][:, 512:1024], AF.Copy,
                         scale=st["sc"][:, 0:1])
    nc.sync.dma_start(cx.out[:, 512:1024], f_sb[:, 512:1024])


def _emit_tail(cx, st):
    _tail_exp(cx, st)
    _tail_recip(cx, st)
    _tail_muls(cx, st)


def _emit_half(cx, cur, prev_st, produce_next):
    """Pipelined half-body: passes+scoring of `cur`, tail of `prev_st`,
    produce of the next logical iteration. Returns (state, nxt)."""
    f32 = cx.f32
    e_ps = cx.epsum.tile([128, 1024], f32, tag="eps")
    st = {"eps": e_ps}
    nxt = _produce_head(cx) if produce_next else None
    if nxt is not None:
        _produce_chunks(cx, nxt, 0, 1)
    _emit_group(cx, cur, 0, e_ps)
    if prev_st is not None:
        _tail_exp(cx, prev_st)
    if nxt is not None:
        _produce_chunks(cx, nxt, 2, 3)
    _emit_group(cx, cur, 1, e_ps)
    if nxt is not None:
        _produce_chunks(cx, nxt, 4, 5)
    _emit_group(cx, cur, 2, e_ps)
    if prev_st is not None:
        _tail_recip(cx, prev_st)
    if nxt is not None:
        _produce_chunks(cx, nxt, 6, 7)
    for g in range(3, NG):
        _emit_group(cx, cur, g, e_ps)
    if prev_st is not None:
        _tail_muls(cx, prev_st)
    return st, nxt


def _body_straight(cx):
    """Single-shot body: produce + groups (g0 split for startup) + tail."""
    cur = _produce_head(cx)
    _produce_chunks(cx, cur, 0, 7)
    f32 = cx.f32
    e_ps = cx.epsum.tile([128, 1024], f32, tag="eps")
    for g in range(NG):
        _emit_group(cx, cur, g, e_ps, split_g0=True)
    _emit_tail(cx, {"eps": e_ps})


def _build(reps=1, loop=False):
    import concourse.mybir as mybir
    import concourse.tile as tile
    from concourse import bacc

    dt = mybir.dt
    f32, bf = dt.float32, dt.bfloat16
    AF = mybir.ActivationFunctionType
    ALU = mybir.AluOpType

    nc = bacc.Bacc("TRN2", target_bir_lowering=False, debug=False,
                   enable_asserts=True)

    xbt = nc.dram_tensor("xbt", [128, L], bf, kind="ExternalInput").ap()
    xat = nc.dram_tensor("xat", [128, KPC], bf, kind="ExternalInput").ap()
    w1a = nc.dram_tensor("w1a", [128, H], bf, kind="ExternalInput").ap()
    w1b = nc.dram_tensor("w1b", [128, H], bf, kind="ExternalInput").ap()
    b1c = nc.dram_tensor("b1c", [128, 1], f32, kind="ExternalInput").ap()
    wband = nc.dram_tensor("wband", [128, 64], bf, kind="ExternalInput").ap()
    wcomb = nc.dram_tensor("wcomb", [128, KPC], f32, kind="ExternalInput").ap()
    wbcast = nc.dram_tensor("wbcast", [KPC, 128], f32, kind="ExternalInput").ap()
    out = nc.dram_tensor("out", [128, 1024], bf, kind="ExternalOutput").ap()

    with tile.TileContext(nc) as tc:
        with (
            tc.tile_pool(name="consts", bufs=1) as consts,
            tc.tile_pool(name="work", bufs=2) as work,
            tc.tile_pool(name="rpool", bufs=10) as rpool,
            tc.tile_pool(name="apool",
                         bufs=max(1, 2 * len(FUSED_PASSES))) as apool,
            tc.tile_pool(name="stage", bufs=9) as stage,
            tc.tile_pool(name="psum", bufs=3, space="PSUM") as psum,
            tc.tile_pool(name="epsum", bufs=2, space="PSUM") as epsum,
        ):
            w1a_sb = consts.tile([128, H], bf, tag="w1a")
            nc.sync.dma_start(w1a_sb[:], w1a)
            w1b_sb = consts.tile([128, H], bf, tag="w1b")
            nc.sync.dma_start(w1b_sb[:], w1b)
            xat_sb = consts.tile([128, KPC], bf, tag="xat")
            nc.sync.dma_start(xat_sb[:], xat)
            b1_sb = consts.tile([128, 1], f32, tag="b1")
            nc.sync.dma_start(b1_sb[:], b1c)
            wband_sb = consts.tile([128, 64], bf, tag="wband")
            nc.sync.dma_start(wband_sb[:], wband)
            wcomb_sb = consts.tile([128, KPC], f32, tag="wcomb")
            nc.sync.dma_start(wcomb_sb[:], wcomb)
            wbcast_sb = consts.tile([KPC, 128], f32, tag="wbcast")
            nc.sync.dma_start(wbcast_sb[:], wbcast)
            # dummy ACT op issued first so the ~2.7us activation-table load
            # overlaps the input DMAs; Exp anchors the exp_and_others table
            # set which also holds Relu and Copy
            warm_sb = consts.tile([128, 1], f32, tag="warm")
            nc.vector.memset(warm_sb[:], 0.0)
            nc.scalar.activation(warm_sb[:], warm_sb[:], AF.Exp)

            cx = _Ctx(nc, tc,
                      (work, rpool, apool, stage, psum, epsum),
                      (xbt, out, w1a_sb, w1b_sb, xat_sb, b1_sb, wband_sb,
                       wcomb_sb, wbcast_sb),
                      (f32, bf, AF, ALU))

            if loop and reps > 1:
                U = LOOP_HALVES
                assert U % 2 == 0 and (reps - 2) % U == 0 and reps >= U + 2, \
                    f"loop path needs reps = 2 + {U}*m"
                m = (reps - 2) // U
                cur0 = _produce_head(cx)                     # logical 0
                _produce_chunks(cx, cur0, 0, 7)
                st_a, cur1 = _emit_half(cx, cur0, None, True)
                with tc.For_i(0, m, 1, staggered_reset=STAGGERED):
                    st, cur = st_a, cur1
                    for _u in range(U):
                        st, cur = _emit_half(cx, cur, st, True)
                st_last, _ = _emit_half(cx, cur1, st_a, False)
                _emit_tail(cx, st_last)
            else:
                for _rep in range(reps):
                    _body_straight(cx)

    nc.compile()
    return nc


def _get_built():
    global _BUILT
    if _BUILT is None:
        _BUILT = _build()
    return _BUILT


def make_in_maps(inputA, inputB, W1, b1, w2):
    wband = np.zeros((128, 64), np.float32)
    wband[:, 31] = w2
    wcomb = (np.arange(128)[:, None] // 4 == np.arange(KPC)[None, :]) \
        .astype(np.float32)
    wbcast = (np.arange(128)[None, :] // 4 == np.arange(KPC)[:, None]) \
        .astype(np.float32)
    w1a = np.ascontiguousarray(W1[:D]).astype(BF)
    w1b = np.ascontiguousarray(W1[D:]).astype(BF)
    b1c = np.ascontiguousarray(b1.reshape(128, 1)).astype(np.float32)
    wband = wband.astype(BF)
    in_maps = []
    for core in range(NCORES):
        b, kq = core // 4, core % 4
        k0 = KPC * kq
        in_maps.append({
            "xbt": np.ascontiguousarray(inputB[b].T).astype(BF),
            "xat": np.ascontiguousarray(inputA[b, k0:k0 + KPC].T).astype(BF),
            "w1a": w1a, "w1b": w1b, "b1c": b1c, "wband": wband,
            "wcomb": wcomb, "wbcast": wbcast,
        })
    return in_maps


def assemble(results):
    """results: list of 8 dicts with 'out' [128, 1024] bf16."""
    full = np.empty((B, K, L), np.float32)
    for core in range(NCORES):
        b, kq = core // 4, core % 4
        full[b, KPC * kq:KPC * (kq + 1)] = \
            np.asarray(results[core]["out"]).astype(np.float32) \
            .reshape(KPC, L)
    return full


def kernel(**inputs):
    from concourse.bass_utils import run_bass_kernel_spmd

    inputA = np.asarray(inputs["inputA"], np.float32)
    inputB = np.asarray(inputs["inputB"], np.float32)
    W1 = np.asarray(inputs["W1"], np.float32)
    b1 = np.asarray(inputs["b1"], np.float32)
    w2 = np.asarray(inputs["w2"], np.float32)

    nc = _get_built()
    in_maps = make_in_maps(inputA, inputB, W1, b1, w2)
    res = run_bass_kernel_spmd(nc, in_maps, core_ids=list(range(NCORES)))
    return assemble(res.results)
